# revision 4
# baseline (speedup 1.0000x reference)
"""Trainium2 Bass kernel for nn_BatchGraphEncoder (gnn_message_passing).

Math note: the reference's segment softmax uses B unique segment ids
(groups of size 1), so alpha == exp(x-x)/1 == 1.0 bit-exactly for any
finite scores.  The output is therefore independent of the attention
inputs (w_i, w_j, w_k) and reduces to pure batch sums:

    out[:,   0:128] = sum_b h[b,:]      (broadcast over the N=512 rows)
    out[:, 128:256] = sum_b r[b,:]      (broadcast)
    out[:, 256:384] = sum_b t[b,:,:]    ([512, 128])

This is a memory-bound reduction over B=2048 dominated by reading t
(512 MB).  Strategy: shard B across the 8 cores (data parallel), reduce
over the local batch on-device, and sum the 8 tiny partials on the host.

Pipeline design (v2): the previous per-tile fold-tree (3 halving folds
+ accumulator merge, ~10.5us of DVE per 4 MB tile) slot-coupled the DMA
stream to the DVE and collapsed the last ~40us of the stream to fold
pace.  Now each tile gets ONE DVE op: tensor_tensor add into a wide
[128, 4096] accumulator (free dim packs 8 batch-row slots x 512 cols).
2 MB tiles / 10 pool buffers keep the DVE ~20% faster than the DMA
stream with a 20 MB elasticity window; a 1 MB-tile tail drains the
backlog, and one 4096->512 fold at the very end produces the partial.

Tile layout: partition p holds flat columns [512p, 512p+512) of the
[B_loc, 65536] shard; the free dim packs NB batch rows.  DMA issue
alternates between the SP and ACT HWDGE rings.

The h/r sums ride on the otherwise-idle TensorEngine: a stationary
matrix whose column j is all-ones places column-sums of the moving
operand into PSUM row j (rows 0/1 = sum_h/sum_r).

Load balancing: cores 4 and 6 of this machine usually lose ~10% DMA
bandwidth (one slow SDMA engine each), so they get smaller shards:
rows [224, 240) are only loaded when partition_id != 6, rows [240, 264)
when partition_id not in {4, 6} (the skipping cores' buffers hold stale
finite data there; a per-partition scalar mask gates those tiles'
accumulator merges, and h/r padding rows are zeros, which is exact).
"""

import numpy as np

B, N, D = 2048, 512, 128
NCORES = 8
FLAT = N * D                 # 65536 flattened (n, d) columns
MMW = 512                    # columns per row-slot in the free dim
ACCW = 8 * MMW               # accumulator free width (8 row slots)

# Cores 6 and 4 intermittently lose ~10% DMA bandwidth, so they get
# smaller shards.  Rows [224, 240) are skipped on core 6, rows
# [240, 264) on cores 4 and 6.
B_FAST = 264
SIZES = [B_FAST] * NCORES
SIZES[4] = 240
SIZES[6] = 224
assert sum(SIZES) == B

# (row0, NB, conditional) in emission order.  Conditional tiles sit
# mid-stream at slot indices >= bufs so they never touch first-use SBUF
# on a skipping core.  The tail shrinks to 1 MB then 0.5 MB tiles whose
# merges (2.3us / 1.2us) sit below their DMA times, so the merge
# backlog drains with the stream; staged pre-folds (slots 4-7 during
# the NB=4 tiles, slots 2-3 during the NB=2 tiles) leave only a
# ~2us fold chain after the last input byte.
_U = [(r, 8, None) for r in range(0, 192, 8)]        # 24 big tiles
_C6 = [(224, 8, "c6"), (232, 8, "c6")]
_C46 = [(240, 8, "c46"), (248, 8, "c46"), (256, 8, "c46")]
_TAIL = [(r, 4, None) for r in range(192, 216, 4)] + [
    (r, 2, None) for r in range(216, 224, 2)
]
TILE_PLAN = (
    _U[0:12]
    + [_C6[0]]
    + _U[12:15]
    + [_C6[1]]
    + _U[15:18]
    + [_C46[0]]
    + _U[18:21]
    + [_C46[1]]
    + _U[21:24]
    + [_C46[2]]
    + _TAIL
)
assert sum(nb for _, nb, _ in TILE_PLAN) == B_FAST
assert sorted(r for r, nb, c in TILE_PLAN) == sorted(
    r for r, nb, c in _U + _C6 + _C46 + _TAIL
)
LAST_NB8 = 28   # emission index of the last NB=8 tile (C46[2])
LAST_NB4 = 34   # emission index of the last NB=4 tile

NBUFS = 10

_BUILT = None
# test.py can inject {"trace": True, ...} here; harness path leaves it empty.
RUN_KWARGS = {}
LAST_RESULTS = None


def _build():
    from concourse import bacc, tile, mybir

    f32 = mybir.dt.float32
    add = mybir.AluOpType.add
    nc = bacc.Bacc(
        "TRN2",
        target_bir_lowering=False,
        debug=False,
        enable_asserts=False,
        num_devices=NCORES,
    )
    t_in = nc.dram_tensor("t_shard", [B_FAST, FLAT], f32, kind="ExternalInput").ap()
    h_in = nc.dram_tensor("h_shard", [B_FAST, D], f32, kind="ExternalInput").ap()
    r_in = nc.dram_tensor("r_shard", [B_FAST, D], f32, kind="ExternalInput").ap()
    out_t = nc.dram_tensor("out_t_part", [128, MMW], f32, kind="ExternalOutput").ap()
    out_hr = nc.dram_tensor("out_hr_part", [2, D], f32, kind="ExternalOutput").ap()

    with tile.TileContext(nc) as tc:
        with (
            tc.tile_pool(name="wconst", bufs=1) as wpool,
            tc.tile_pool(name="loads", bufs=NBUFS) as loads,
            tc.tile_pool(name="hr", bufs=6) as hrpool,
            tc.tile_pool(name="res", bufs=1) as res,
            tc.tile_pool(name="acc", bufs=1, space="PSUM") as ppool,
        ):
            W = wpool.tile([128, 256], f32)
            mask6 = wpool.tile([128, 1], f32)
            mask46 = wpool.tile([128, 1], f32)
            psum_hr = ppool.tile([128, D], f32)
            acc = res.tile([128, ACCW], f32)
            res_hr = res.tile([2, D], f32)
            skip_cond = {}
            masks = {"c6": mask6, "c46": mask46}

            def emit_setup_and_hr():
                # Emitted after the first few t loads so the pid register
                # loads and h/r DMAs never delay the t stream's start; h/r
                # loads ride the SWDGE (gpsimd) ring, keeping both HWDGE
                # rings exclusively on t tiles.
                # W is zero except column 128 == 1.0; W[:, 128-j : 256-j]
                # is a [128, 128] stationary whose column j is all-ones.
                nc.vector.memset(W[:], 0.0)
                nc.vector.memset(W[:, 128:129], 1.0)
                # mask6/mask46 = 0.0 on the core(s) that skip that tier,
                # 1.0 elsewhere; they gate the accumulator merges of the
                # conditional tiles.
                nc.vector.memset(mask6[:], 1.0)
                nc.vector.memset(mask46[:], 1.0)
                pid_vec = nc.vector.partition_id()
                with tc.If(pid_vec == 6):
                    nc.vector.memset(mask6[:], 0.0)
                    nc.vector.memset(mask46[:], 0.0)
                with tc.If(pid_vec == 4):
                    nc.vector.memset(mask46[:], 0.0)
                pid_sync = nc.sync.partition_id()
                pid_act = nc.scalar.partition_id()
                skip_cond["c6"] = {
                    nc.sync: pid_sync != 6,
                    nc.scalar: pid_act != 6,
                }
                skip_cond["c46"] = {
                    nc.sync: (pid_sync != 6) * (pid_sync != 4),
                    nc.scalar: (pid_act != 6) * (pid_act != 4),
                }

                # h / r batch sums -> rows 0 / 1 of psum_hr
                # (padding rows on short-shard cores are zeros; exact)
                chunks = []
                for row, src in ((0, h_in), (1, r_in)):
                    for c0 in range(0, B_FAST, 128):
                        k = min(128, B_FAST - c0)
                        ht = hrpool.tile([128, D], f32)
                        nc.gpsimd.dma_start(ht[:k, :], src[c0 : c0 + k, :])
                        chunks.append((row, ht, k))
                for i, (row, ht, k) in enumerate(chunks):
                    nc.tensor.matmul(
                        psum_hr[:],
                        W[:k, 128 - row : 256 - row],
                        ht[:k, :],
                        start=(i == 0),
                        stop=(i == len(chunks) - 1),
                    )
                # Ship the h/r partial mid-stream, off the tail.
                nc.vector.tensor_copy(res_hr[:], psum_hr[0:2, :])
                nc.sync.dma_start(out_hr[:], res_hr[:])

            # --- t batch sum: one DVE merge per tile into acc ---
            for k, (b0, NB, cnd) in enumerate(TILE_PLAN):
                if k == 2:
                    emit_setup_and_hr()
                fw = NB * MMW  # free width
                tl = loads.tile([128, 8 * MMW], f32)
                src = t_in[b0 : b0 + NB, :].rearrange("b (p c) -> p b c", p=128)
                dma = nc.sync if k % 2 == 0 else nc.scalar
                dst = tl[:, :fw].rearrange("p (b c) -> p b c", b=NB)
                if cnd:
                    # Skipped on the slow core(s): the slot then holds stale
                    # (finite) data from an earlier tile; the masked merge
                    # zeroes it.
                    dma.dma_start(dst, src, cond=skip_cond[cnd][dma])
                else:
                    dma.dma_start(dst, src)
                if k == 0:
                    nc.vector.tensor_copy(acc[:], tl[:, :ACCW])
                elif cnd:
                    # acc = (tile * mask) + acc
                    nc.vector.scalar_tensor_tensor(
                        acc[:, :fw],
                        tl[:, :fw],
                        masks[cnd][:],
                        acc[:, :fw],
                        mybir.AluOpType.mult,
                        add,
                    )
                else:
                    nc.vector.tensor_tensor(acc[:, :fw], acc[:, :fw], tl[:, :fw], add)
                if k == LAST_NB8:
                    # slots 4-7 are final; fold them while NB=4 tiles stream:
                    # cols [2048, 3072) := s4+s6 | s5+s7
                    nc.vector.tensor_tensor(
                        acc[:, 2048:3072], acc[:, 2048:3072], acc[:, 3072:4096], add
                    )
                if k == LAST_NB4:
                    # slots 2-3 final; fold in s4..s7 while NB=2 tiles stream:
                    # cols [1024, 2048) := s2+s4+s6 | s3+s5+s7
                    nc.vector.tensor_tensor(
                        acc[:, 1024:2048], acc[:, 1024:2048], acc[:, 2048:3072], add
                    )

            # Final fold chain after the last merge: 1024 + 512 elems.
            nc.vector.tensor_tensor(
                acc[:, :1024], acc[:, :1024], acc[:, 1024:2048], add
            )
            nc.vector.tensor_tensor(acc[:, :512], acc[:, :512], acc[:, 512:1024], add)
            nc.sync.dma_start(out_t[:], acc[:, :MMW])

    nc.compile()
    return nc


def _get_built():
    global _BUILT
    if _BUILT is None:
        _BUILT = _build()
    return _BUILT


def kernel(h, r, t, w_i, w_j, w_k):
    global LAST_RESULTS
    from concourse import bass_utils

    nc = _get_built()
    t2 = np.ascontiguousarray(t, dtype=np.float32).reshape(B, FLAT)
    h = np.ascontiguousarray(h, dtype=np.float32)
    r = np.ascontiguousarray(r, dtype=np.float32)

    def pad(a, ncols):
        out = np.zeros((B_FAST, ncols), dtype=np.float32)
        out[: a.shape[0]] = a
        return out

    starts = np.concatenate([[0], np.cumsum(SIZES)])
    in_maps = []
    for c in range(NCORES):
        s, e = int(starts[c]), int(starts[c + 1])
        if e - s == B_FAST:
            in_maps.append({"t_shard": t2[s:e], "h_shard": h[s:e], "r_shard": r[s:e]})
        else:
            in_maps.append(
                {
                    "t_shard": pad(t2[s:e], FLAT),
                    "h_shard": pad(h[s:e], D),
                    "r_shard": pad(r[s:e], D),
                }
            )
    results = bass_utils.run_bass_kernel_spmd(
        nc, in_maps, core_ids=list(range(NCORES)), **RUN_KWARGS
    )
    LAST_RESULTS = results

    sum_t = np.zeros(FLAT, dtype=np.float64)
    sum_h = np.zeros(D, dtype=np.float64)
    sum_r = np.zeros(D, dtype=np.float64)
    for c in range(NCORES):
        sum_t += results.results[c]["out_t_part"].reshape(FLAT)
        sum_h += results.results[c]["out_hr_part"][0]
        sum_r += results.results[c]["out_hr_part"][1]

    out = np.empty((N, 3 * D), dtype=np.float32)
    out[:, 0:D] = sum_h.astype(np.float32)[None, :]
    out[:, D : 2 * D] = sum_r.astype(np.float32)[None, :]
    out[:, 2 * D :] = sum_t.astype(np.float32).reshape(N, D)
    return out


# revision 6
# speedup vs baseline: 1.0480x; 1.0480x over previous
"""Trainium2 Bass kernel for nn_BatchGraphEncoder (gnn_message_passing).

Math note: the reference's segment softmax uses B unique segment ids
(groups of size 1), so alpha == exp(x-x)/1 == 1.0 bit-exactly for any
finite scores.  The output is therefore independent of the attention
inputs (w_i, w_j, w_k) and reduces to pure batch sums:

    out[:,   0:128] = sum_b h[b,:]      (broadcast over the N=512 rows)
    out[:, 128:256] = sum_b r[b,:]      (broadcast)
    out[:, 256:384] = sum_b t[b,:,:]    ([512, 128])

This is a memory-bound reduction over B=2048 dominated by reading t
(512 MB).  Strategy: shard B across the 8 cores (data parallel), reduce
over the local batch on-device, and sum the 8 tiny partials on the host.

Pipeline design (v2): the previous per-tile fold-tree (3 halving folds
+ accumulator merge, ~10.5us of DVE per 4 MB tile) slot-coupled the DMA
stream to the DVE and collapsed the last ~40us of the stream to fold
pace.  Now each tile gets ONE DVE op: tensor_tensor add into a wide
[128, 4096] accumulator (free dim packs 8 batch-row slots x 512 cols).
2 MB tiles / 10 pool buffers keep the DVE ~20% faster than the DMA
stream with a 20 MB elasticity window; a 1 MB-tile tail drains the
backlog, and one 4096->512 fold at the very end produces the partial.

Tile layout: partition p holds flat columns [512p, 512p+512) of the
[B_loc, 65536] shard; the free dim packs NB batch rows.  DMA issue
alternates between the SP and ACT HWDGE rings.

The h/r sums ride on the otherwise-idle TensorEngine: a stationary
matrix whose column j is all-ones places column-sums of the moving
operand into PSUM row j (rows 0/1 = sum_h/sum_r).

Load balancing: cores 4 and 6 of this machine usually lose ~10% DMA
bandwidth (one slow SDMA engine each), so they get smaller shards:
rows [224, 240) are only loaded when partition_id != 6, rows [240, 264)
when partition_id not in {4, 6} (the skipping cores' buffers hold stale
finite data there; a per-partition scalar mask gates those tiles'
accumulator merges, and h/r padding rows are zeros, which is exact).
"""

import numpy as np

B, N, D = 2048, 512, 128
NCORES = 8
FLAT = N * D                 # 65536 flattened (n, d) columns
MMW = 512                    # columns per row-slot in the free dim
ACCW = 8 * MMW               # accumulator free width (8 row slots)

# Cores 6 and 4 intermittently lose ~10% DMA bandwidth, so they get
# smaller shards.  Rows [224, 240) are skipped on core 6, rows
# [240, 264) on cores 4 and 6.
B_FAST = 264
SIZES = [B_FAST] * NCORES
SIZES[4] = 240
SIZES[6] = 224
assert sum(SIZES) == B

# (row0, NB, conditional) in emission order.  The stream is shaped so
# the in-order DVE merge chain never trails the stream end: conditional
# big tiles sit right after the 10-tile slot warm-up (their slots are
# initialized, and their 4.5us masked merges land early), the remaining
# big tiles finish ~30us before the stream does, and the tail tapers to
# 1 MB then 0.5 MB tiles whose merges (2.3us / 1.2us) sit below their
# DMA times so the DVE rides the stream out with zero lag.  Staged
# pre-folds (slots 4-7 during the NB=4 tiles, slots 2-3 during the NB=2
# tiles) leave only a ~2us fold chain after the last input byte.
_UA = [(r, 8, None) for r in range(0, 80, 8)]        # 10 warm-up tiles
_C6 = [(224, 8, "c6"), (232, 8, "c6")]
_C46 = [(240, 8, "c46"), (248, 8, "c46"), (256, 8, "c46")]
_UB = [(r, 8, None) for r in range(80, 176, 8)]      # 12 big tiles
_T4 = [(r, 4, None) for r in range(176, 208, 4)]     # 8 x 1 MB
_T2 = [(r, 2, None) for r in range(208, 224, 2)]     # 8 x 0.5 MB
TILE_PLAN = _UA + _C6 + _C46 + _UB + _T4 + _T2
assert sum(nb for _, nb, _ in TILE_PLAN) == B_FAST
assert sorted(r for r, nb, c in TILE_PLAN) == sorted(
    r for r, nb, c in _UA + _C6 + _C46 + _UB + _T4 + _T2
)
LAST_NB8 = 26   # emission index of the last NB=8 tile (last of _UB)
LAST_NB4 = 34   # emission index of the last NB=4 tile

NBUFS = 10

_BUILT = None
# test.py can inject {"trace": True, ...} here; harness path leaves it empty.
RUN_KWARGS = {}
LAST_RESULTS = None


def _build():
    from concourse import bacc, tile, mybir

    f32 = mybir.dt.float32
    add = mybir.AluOpType.add
    nc = bacc.Bacc(
        "TRN2",
        target_bir_lowering=False,
        debug=False,
        enable_asserts=False,
        num_devices=NCORES,
    )
    t_in = nc.dram_tensor("t_shard", [B_FAST, FLAT], f32, kind="ExternalInput").ap()
    h_in = nc.dram_tensor("h_shard", [B_FAST, D], f32, kind="ExternalInput").ap()
    r_in = nc.dram_tensor("r_shard", [B_FAST, D], f32, kind="ExternalInput").ap()
    out_t = nc.dram_tensor("out_t_part", [128, MMW], f32, kind="ExternalOutput").ap()
    out_hr = nc.dram_tensor("out_hr_part", [2, D], f32, kind="ExternalOutput").ap()

    with tile.TileContext(nc) as tc:
        with (
            tc.tile_pool(name="wconst", bufs=1) as wpool,
            tc.tile_pool(name="loads", bufs=NBUFS) as loads,
            tc.tile_pool(name="hr", bufs=6) as hrpool,
            tc.tile_pool(name="res", bufs=1) as res,
            tc.tile_pool(name="acc", bufs=1, space="PSUM") as ppool,
        ):
            W = wpool.tile([128, 256], f32)
            mask6 = wpool.tile([128, 1], f32)
            mask46 = wpool.tile([128, 1], f32)
            psum_hr = ppool.tile([128, D], f32)
            acc = res.tile([128, ACCW], f32)
            res_hr = res.tile([2, D], f32)
            skip_cond = {}
            masks = {"c6": mask6, "c46": mask46}

            def emit_setup_and_hr():
                # Emitted after the first few t loads so the pid register
                # loads and h/r DMAs never delay the t stream's start; h/r
                # loads ride the SWDGE (gpsimd) ring, keeping both HWDGE
                # rings exclusively on t tiles.
                # W is zero except column 128 == 1.0; W[:, 128-j : 256-j]
                # is a [128, 128] stationary whose column j is all-ones.
                nc.vector.memset(W[:], 0.0)
                nc.vector.memset(W[:, 128:129], 1.0)
                # mask6/mask46 = 0.0 on the core(s) that skip that tier,
                # 1.0 elsewhere; they gate the accumulator merges of the
                # conditional tiles.
                nc.vector.memset(mask6[:], 1.0)
                nc.vector.memset(mask46[:], 1.0)
                pid_vec = nc.vector.partition_id()
                with tc.If(pid_vec == 6):
                    nc.vector.memset(mask6[:], 0.0)
                    nc.vector.memset(mask46[:], 0.0)
                with tc.If(pid_vec == 4):
                    nc.vector.memset(mask46[:], 0.0)
                pid_sync = nc.sync.partition_id()
                pid_act = nc.scalar.partition_id()
                skip_cond["c6"] = {
                    nc.sync: pid_sync != 6,
                    nc.scalar: pid_act != 6,
                }
                skip_cond["c46"] = {
                    nc.sync: (pid_sync != 6) * (pid_sync != 4),
                    nc.scalar: (pid_act != 6) * (pid_act != 4),
                }

                # h / r batch sums -> rows 0 / 1 of psum_hr
                # (padding rows on short-shard cores are zeros; exact)
                chunks = []
                for row, src in ((0, h_in), (1, r_in)):
                    for c0 in range(0, B_FAST, 128):
                        k = min(128, B_FAST - c0)
                        ht = hrpool.tile([128, D], f32)
                        nc.gpsimd.dma_start(ht[:k, :], src[c0 : c0 + k, :])
                        chunks.append((row, ht, k))
                for i, (row, ht, k) in enumerate(chunks):
                    nc.tensor.matmul(
                        psum_hr[:],
                        W[:k, 128 - row : 256 - row],
                        ht[:k, :],
                        start=(i == 0),
                        stop=(i == len(chunks) - 1),
                    )
                # Ship the h/r partial mid-stream, off the tail.
                nc.vector.tensor_copy(res_hr[:], psum_hr[0:2, :])
                nc.sync.dma_start(out_hr[:], res_hr[:])

            # --- t batch sum: one DVE merge per tile into acc ---
            ring_bytes = [0, 0]  # greedy byte-balance across the 2 HWDGE rings
            for k, (b0, NB, cnd) in enumerate(TILE_PLAN):
                if k == 2:
                    emit_setup_and_hr()
                fw = NB * MMW  # free width
                tl = loads.tile([128, 8 * MMW], f32)
                src = t_in[b0 : b0 + NB, :].rearrange("b (p c) -> p b c", p=128)
                ring = (
                    (k % 2)
                    if ring_bytes[0] == ring_bytes[1]
                    else int(ring_bytes[1] < ring_bytes[0])
                )
                ring_bytes[ring] += NB
                dma = nc.sync if ring == 0 else nc.scalar
                dst = tl[:, :fw].rearrange("p (b c) -> p b c", b=NB)
                if cnd:
                    # Skipped on the slow core(s): the slot then holds stale
                    # (finite) data from an earlier tile; the masked merge
                    # zeroes it.
                    dma.dma_start(dst, src, cond=skip_cond[cnd][dma])
                else:
                    dma.dma_start(dst, src)
                if k == 0:
                    nc.vector.tensor_copy(acc[:], tl[:, :ACCW])
                elif cnd:
                    # acc = (tile * mask) + acc
                    nc.vector.scalar_tensor_tensor(
                        acc[:, :fw],
                        tl[:, :fw],
                        masks[cnd][:],
                        acc[:, :fw],
                        mybir.AluOpType.mult,
                        add,
                    )
                else:
                    nc.vector.tensor_tensor(acc[:, :fw], acc[:, :fw], tl[:, :fw], add)
                if k == LAST_NB8:
                    # slots 4-7 are final; fold them while NB=4 tiles stream:
                    # cols [2048, 3072) := s4+s6 | s5+s7
                    nc.vector.tensor_tensor(
                        acc[:, 2048:3072], acc[:, 2048:3072], acc[:, 3072:4096], add
                    )
                if k == LAST_NB4:
                    # slots 2-3 final; fold in s4..s7 while NB=2 tiles stream:
                    # cols [1024, 2048) := s2+s4+s6 | s3+s5+s7
                    nc.vector.tensor_tensor(
                        acc[:, 1024:2048], acc[:, 1024:2048], acc[:, 2048:3072], add
                    )

            # Final fold chain after the last merge: 1024 + 512 elems.
            nc.vector.tensor_tensor(
                acc[:, :1024], acc[:, :1024], acc[:, 1024:2048], add
            )
            nc.vector.tensor_tensor(acc[:, :512], acc[:, :512], acc[:, 512:1024], add)
            nc.sync.dma_start(out_t[:], acc[:, :MMW])

    nc.compile()
    return nc


def _get_built():
    global _BUILT
    if _BUILT is None:
        _BUILT = _build()
    return _BUILT


def kernel(h, r, t, w_i, w_j, w_k):
    global LAST_RESULTS
    from concourse import bass_utils

    nc = _get_built()
    t2 = np.ascontiguousarray(t, dtype=np.float32).reshape(B, FLAT)
    h = np.ascontiguousarray(h, dtype=np.float32)
    r = np.ascontiguousarray(r, dtype=np.float32)

    def pad(a, ncols):
        out = np.zeros((B_FAST, ncols), dtype=np.float32)
        out[: a.shape[0]] = a
        return out

    starts = np.concatenate([[0], np.cumsum(SIZES)])
    in_maps = []
    for c in range(NCORES):
        s, e = int(starts[c]), int(starts[c + 1])
        if e - s == B_FAST:
            in_maps.append({"t_shard": t2[s:e], "h_shard": h[s:e], "r_shard": r[s:e]})
        else:
            in_maps.append(
                {
                    "t_shard": pad(t2[s:e], FLAT),
                    "h_shard": pad(h[s:e], D),
                    "r_shard": pad(r[s:e], D),
                }
            )
    results = bass_utils.run_bass_kernel_spmd(
        nc, in_maps, core_ids=list(range(NCORES)), **RUN_KWARGS
    )
    LAST_RESULTS = results

    sum_t = np.zeros(FLAT, dtype=np.float64)
    sum_h = np.zeros(D, dtype=np.float64)
    sum_r = np.zeros(D, dtype=np.float64)
    for c in range(NCORES):
        sum_t += results.results[c]["out_t_part"].reshape(FLAT)
        sum_h += results.results[c]["out_hr_part"][0]
        sum_r += results.results[c]["out_hr_part"][1]

    out = np.empty((N, 3 * D), dtype=np.float32)
    out[:, 0:D] = sum_h.astype(np.float32)[None, :]
    out[:, D : 2 * D] = sum_r.astype(np.float32)[None, :]
    out[:, 2 * D :] = sum_t.astype(np.float32).reshape(N, D)
    return out
